# revision 7
# baseline (speedup 1.0000x reference)
"""Trainium2 Bass kernel for ExpanderLinear: out = x @ (W * mask).T

Shapes (hardcoded): x [8192, 4096] f32, weight [4096, 4096] f32,
mask [4096, 4096] f32 -> out [8192, 4096] f32.

Strategy: tensor-parallel over output features across 8 cores. The host
pre-marshals operands (like GEMM pre-packing): wm = (W*mask)*32
premultiplied, transposed, and split along the contraction dim:
  - rows 0..3583  -> bf16   (28 of 32 contraction chunks)
  - rows 3584..4095 -> fp8e4m3, computed with DoubleRow matmuls
    (2 contraction chunks of 256 per instruction, 2 MACs/cell/cycle)
x is transposed and split the same way (bf16 + fp8). The *32 weight
scale (exact in bf16, keeps fp8 weights out of the subnormal range) is
undone by the PSUM-drain copy (tensor_scalar_mul 1/32). Measured absmax
error vs the f64 reference: 1.80e-2 of scale (tolerance 2e-2); the
bf16-only variant measures 2.45e-3.

Per-core device kernel:
  - PE warmup matmuls on a memset tile run during the initial DMA wait
    so the HAM clock gate is at 2.4 GHz when data lands.
  - weights persist in SBUF (3.5 MB bf16 + 0.25 MB fp8), loaded
    interleaved with the first chunk's x loads (chunk 0 uses 512-col
    half tiles so the first matmul's dependencies are only ~1 MB).
  - loop over 8 batch chunks of 1024 (bf16 DMA tiles [128, 4, 1024],
    2 KB/partition lines), each split into two 512-wide matmul halves:
    4 psum banks per half; per oc 28 bf16 matmuls + 2 fp8 DoubleRow
    matmuls accumulate, then DVE drains with the 1/32 scale and the
    result is DMA'd out. Prefetch for chunk c+1 is spread through c's
    second half for ~25 us of DMA lead time.
"""

import ml_dtypes
import numpy as np

import concourse.bass as bass
import concourse.mybir as mybir
import concourse.tile as tile
from concourse import bacc
from concourse.bass_utils import run_bass_kernel_spmd

P = 128
D_IN = 4096
D_OUT = 4096
BATCH = 8192
N_CORES = 8
O_PER_CORE = D_OUT // N_CORES  # 512
OT = O_PER_CORE // P  # 4 output partition tiles
KCG = 4  # contraction chunks (of 128) per bf16 group
K8_PAIRS = 2  # fp8 DoubleRow matmuls per oc (each covers 256 of K)
K8 = K8_PAIRS * 2 * P  # 512 contraction rows in fp8
KB = D_IN - K8  # 3584 contraction rows in bf16
KGB = KB // (KCG * P)  # 7 bf16 groups
BC_DMA = 1024  # batch columns per DMA tile (2 KB bf16 lines)
N_CHUNK = BATCH // BC_DMA  # 8
BN = 512  # matmul free dim (one psum bank)
N_WARM = 32  # PE warmup matmuls (N=128)
WSCALE = 32.0  # host weight pre-scale, undone in the drain

F32 = mybir.dt.float32
BF16 = mybir.dt.bfloat16
F8 = mybir.dt.float8e4
DR = mybir.MatmulPerfMode.DoubleRow


def build_nc():
    nc = bacc.Bacc("TRN2", target_bir_lowering=False, debug=False, num_devices=N_CORES)

    xT_d = nc.dram_tensor("xT", [KB, BATCH], BF16, kind="ExternalInput")
    x8T_d = nc.dram_tensor("x8T", [K8, BATCH], F8, kind="ExternalInput")
    wmT_d = nc.dram_tensor("wmT", [KB, O_PER_CORE], BF16, kind="ExternalInput")
    wm8T_d = nc.dram_tensor("wm8T", [K8, O_PER_CORE], F8, kind="ExternalInput")
    outT_d = nc.dram_tensor("outT", [O_PER_CORE, BATCH], F32, kind="ExternalOutput")

    with tile.TileContext(nc) as tc:
        with (
            tc.tile_pool(name="persist", bufs=1) as persist,
            tc.tile_pool(name="xs", bufs=16) as xspool,
            tc.tile_pool(name="outp", bufs=8) as outp,
            tc.tile_pool(name="mpsum", bufs=8, space="PSUM") as mpsum,
        ):
            # --- PE warmup: emitted first so the tensor queue starts on
            # them while the first DMAs are in flight ---
            wtile = persist.tile([P, P], BF16, name="warm_in")
            nc.gpsimd.memset(wtile, 0)
            wpsum = mpsum.tile([P, BN], F32, name="warm_ps", tag="ps")
            for _ in range(N_WARM):
                nc.tensor.matmul(
                    wpsum[:, 0:P], wtile, wtile, start=True, stop=True
                )

            # --- weight loads, interleaved with the first chunk's x.
            # Weights and chunk-0 first-half x come as 2-ic subtiles so the
            # first matmul depends on only ~0.5 MB; the first few critical
            # loads go out on the scalar engine's HWDGE ring to avoid
            # queueing behind the bulk on the sync ring. ---
            wm_s = []

            def emit_wm_sub(s, eng):
                r_sl = slice(s * 2 * P, (s + 1) * 2 * P)
                wm = persist.tile([P, 2, O_PER_CORE], BF16, name=f"wmT{s}")
                eng.dma_start(
                    wm, wmT_d[r_sl, :].rearrange("(kc p) o -> p kc o", p=P)
                )
                wm_s.append(wm)

            def emit_x_sub(g, k2, eng):
                """chunk-0 h0 bf16 x subtile: 2 ics x 512 cols."""
                xs = xspool.tile([P, 2, BN], BF16, tag="xs", name="xs")
                r0 = g * KCG * P + k2 * 2 * P
                eng.dma_start(
                    xs,
                    xT_d[r0 : r0 + 2 * P, 0:BN].rearrange(
                        "(kc p) b -> p kc b", p=P
                    ),
                )
                return xs

            def emit_x_group(ch, g, h=None):
                """bf16 x group tile; h=None: 1024 cols, h=0/1: 512 cols."""
                cols_n = BC_DMA if h is None else BN
                xs = xspool.tile([P, KCG, cols_n], BF16, tag="xs", name="xs")
                rows = slice(g * KCG * P, (g + 1) * KCG * P)
                c0 = ch * BC_DMA + (0 if h is None else h * BN)
                nc.sync.dma_start(
                    xs,
                    xT_d[rows, c0 : c0 + cols_n].rearrange(
                        "(kc p) b -> p kc b", p=P
                    ),
                )
                return xs

            def emit_x8(ch, h=None):
                """fp8 x tile [P, pairs, 2, cols] in DoubleRow pairing."""
                cols_n = BC_DMA if h is None else BN
                xs = xspool.tile(
                    [P, K8_PAIRS, 2, cols_n], F8, tag="xs", name="xs8"
                )
                c0 = ch * BC_DMA + (0 if h is None else h * BN)
                nc.sync.dma_start(
                    xs,
                    x8T_d[:, c0 : c0 + cols_n].rearrange(
                        "(kp ko p) b -> p kp ko b", p=P, ko=2
                    ),
                )
                return xs

            x0 = {}
            x0h0_sub = []
            for s in range(2 * KGB):
                eng = nc.scalar if s < 2 else nc.sync
                emit_wm_sub(s, eng)
                x0h0_sub.append(emit_x_sub(s // 2, s % 2, eng))
            wm8 = persist.tile([P, K8_PAIRS, 2, O_PER_CORE], F8, name="wm8T")
            nc.sync.dma_start(
                wm8, wm8T_d.rearrange("(kp ko p) o -> p kp ko o", p=P, ko=2)
            )
            x0[(0, KGB)] = emit_x8(0, h=0)
            for g in range(KGB):
                x0[(1, g)] = emit_x_group(0, g, h=1)
            x0[(1, KGB)] = emit_x8(0, h=1)

            def lhsT(ic, oc):
                return wm_s[ic // 2][:, ic % 2, oc * P : (oc + 1) * P]

            def lhsT8(kp, oc):
                return wm8[:, kp, :, oc * P : (oc + 1) * P]

            def drain(psum, oc, ch, h):
                ob = outp.tile([P, BN], F32)
                nc.vector.tensor_scalar_mul(ob, psum, 1.0 / WSCALE)
                b0 = ch * BC_DMA + h * BN
                nc.sync.dma_start(
                    outT_d[oc * P : (oc + 1) * P, b0 : b0 + BN], ob
                )

            # --- main loop over batch chunks ---
            pending = None
            for ch in range(N_CHUNK):
                xs_g = pending
                for h in range(2):
                    def rhs(g, k):
                        if ch == 0:
                            if h == 0:
                                return x0h0_sub[g * 2 + k // 2][:, k % 2, :]
                            return x0[(1, g)][:, k, :]
                        return xs_g[g][:, k, h * BN : (h + 1) * BN]

                    def rhs8(kp):
                        if ch == 0:
                            return x0[(h, KGB)][:, kp, :, :]
                        return xs_g[KGB][:, kp, :, h * BN : (h + 1) * BN]

                    last = ch == N_CHUNK - 1 and h == 1
                    psums = [
                        mpsum.tile([P, BN], F32, name=f"ps{oc}", tag="ps")
                        for oc in range(OT)
                    ]

                    def emit_oc_mms(oc):
                        for g in range(KGB):
                            for k in range(KCG):
                                ic = g * KCG + k
                                nc.tensor.matmul(
                                    psums[oc],
                                    lhsT(ic, oc),
                                    rhs(g, k),
                                    start=(ic == 0),
                                    stop=False,
                                )
                        for kp in range(K8_PAIRS):
                            nc.tensor.matmul(
                                psums[oc],
                                lhsT8(kp, oc),
                                rhs8(kp),
                                start=False,
                                stop=(kp == K8_PAIRS - 1),
                                perf_mode=DR,
                            )

                    if last:
                        # oc-major so each psum finishes early and its
                        # drain + output DMA overlap remaining matmuls
                        for oc in range(OT):
                            emit_oc_mms(oc)
                            drain(psums[oc], oc, ch, h)
                        continue
                    prefetch = []
                    for g in range(KGB):
                        for k in range(KCG):
                            ic = g * KCG + k
                            for oc in range(OT):
                                nc.tensor.matmul(
                                    psums[oc],
                                    lhsT(ic, oc),
                                    rhs(g, k),
                                    start=(ic == 0),
                                    stop=False,
                                )
                        if h == 1 and ch + 1 < N_CHUNK:
                            # spread next-chunk prefetch through this half
                            prefetch.append(emit_x_group(ch + 1, g))
                    for kp in range(K8_PAIRS):
                        for oc in range(OT):
                            nc.tensor.matmul(
                                psums[oc],
                                lhsT8(kp, oc),
                                rhs8(kp),
                                start=False,
                                stop=(kp == K8_PAIRS - 1),
                                perf_mode=DR,
                            )
                    if h == 1 and ch + 1 < N_CHUNK:
                        prefetch.append(emit_x8(ch + 1))
                        pending = prefetch
                    for oc in range(OT):
                        drain(psums[oc], oc, ch, h)

    nc.compile()
    return nc


_NC_CACHE = None


def _shard_inputs(x, weight, mask):
    """Host-side marshalling: premultiply mask, scale by 32, transpose,
    split the contraction dim into bf16 and fp8 parts, slice per core."""
    x = np.asarray(x, dtype=np.float32)
    weight = np.asarray(weight, dtype=np.float32)
    mask = np.asarray(mask, dtype=np.float32)
    xT = x.T
    xT_b = np.ascontiguousarray(xT[:KB].astype(ml_dtypes.bfloat16))
    xT_8 = np.ascontiguousarray(xT[KB:].astype(ml_dtypes.float8_e4m3))
    wsT = ((weight * mask) * np.float32(WSCALE)).T
    in_maps = []
    for c in range(N_CORES):
        sl = slice(c * O_PER_CORE, (c + 1) * O_PER_CORE)
        in_maps.append(
            {
                "xT": xT_b,
                "x8T": xT_8,
                "wmT": np.ascontiguousarray(
                    wsT[:KB, sl].astype(ml_dtypes.bfloat16)
                ),
                "wm8T": np.ascontiguousarray(
                    wsT[KB:, sl].astype(ml_dtypes.float8_e4m3)
                ),
            }
        )
    return in_maps


def kernel(x, weight, mask):
    global _NC_CACHE
    if _NC_CACHE is None:
        _NC_CACHE = build_nc()
    nc = _NC_CACHE

    in_maps = _shard_inputs(x, weight, mask)
    res = run_bass_kernel_spmd(nc, in_maps, core_ids=list(range(N_CORES)))

    out = np.empty((BATCH, D_OUT), dtype=np.float32)
    for c in range(N_CORES):
        sl = slice(c * O_PER_CORE, (c + 1) * O_PER_CORE)
        out[:, sl] = res.results[c]["outT"].T
    return out
